# revision 23
# baseline (speedup 1.0000x reference)
"""Trainium2 Bass kernel for nn_CompositeEmbeddingA (octree composite embedding).

Per sample (1 sample per NeuronCore, batch=8 over 8 cores):
  layers 0-2 (depths 1-3): x = val_emb[v] + pos0[p0] + pos1[p1] + pos2[p2] + dep_emb[d]
  layers 3-4: same sum w/o dep, then Conv1d(E,E,kernel=stride=k), k=4 (l3) / 8 (l4)

Formulation: every layer is  out = MultiHot^T @ Table  on the PE, with the conv
folded into the tables host-side (per tap j, T_j = table @ w[:,:,j].T). The
multi-hot selector matrices are built host-side directly from the integer
indices (pure index preprocessing) and DMA'd in as fp8 (0/1 exact), so the
device spends PE cycles only on the main gather-matmuls — no on-chip one-hot
construction at all.

Row trimming (vs the padded-table formulation):
  - row 0 of each val/pos table is the zero padding row and indices are >= 1
    by construction, so those rows are dropped (a missing one-hot row
    contributes 0, which equals the zero row's contribution either way).
  - conv bias is folded into the 3 val rows of tap 0 (exactly one val row
    fires per token since value >= 1; the bias is all-zero in this problem
    anyway, so a value of 0 would still be handled correctly).
  - depth embeddings keep their own (indexed) rows: 198 rows per B layer.
This gives B=594 rows/5 chunks, L3=768/6, L4=1536/12
(vs 5/7/13 before). Tables are stored fp8 as scaled hi/lo pairs and every
main matmul runs in fp8 DoubleRow perf mode (0.5 cycles/row): the one-hot lhsT
is read through a stride-0 broadcast AP (subrow pairs share the fire bit), the
rhs supplies the hi and lo table halves, and the PSUM->SBUF eviction rescales
by 1/TBL_SCALE. Output is stored bf16 on-device and upcast to f32 on host.
"""

import sys

for _p in ("/opt/trn_rl_repo",):
    if _p not in sys.path:
        sys.path.insert(0, _p)

import numpy as np
import ml_dtypes

RES = 32
SPATIAL = 3
NUM_VOCAB = 3
E = 256
BATCH = 8
LAYER_SIZES = (8, 64, 512, 4096, 32768)
CONV_SIZE = {3: 4, 4: 8}
S_TOTAL = sum(LAYER_SIZES)  # 37448
OUT_TOKENS = 8 + 64 + 512 + 1024 + 4096  # 5704

_BF16 = ml_dtypes.bfloat16
_FP8 = ml_dtypes.float8_e4m3

# virtual layers: (name, out_tokens, out_offset, n_chunks, n_rows)
VLAYERS = (
    ("B", 584, 0, 5, 594),
    ("L3", 1024, 584, 6, 768),
    ("L4", 4096, 1608, 12, 1536),
)
NCH = 5 + 6 + 12  # 23
CHUNK0 = {"B": 0, "L3": 5, "L4": 11}
TBL_SCALE = 64.0  # lifts fp8 hi/lo table entries out of the subnormal range

# multi-hot selectors are streamed in token blocks so PE can start before the
# full 6.3MB L4 selector has landed
L4_BLOCKS = 8
L3_BLOCKS = 2
N_WARMUP = 13  # dummy PE matmuls to cover the initial DMA latency + p-state ramp


def _layer_slices():
    out = []
    start = 0
    for n in LAYER_SIZES:
        out.append((start, start + n))
        start += n
    return out


LAYER_SL = _layer_slices()


def _build_tables(params):
    """Fold conv weights/bias + depth embeddings into per-row tables.

    Returns tbl [128, NCH*2E] fp8: per chunk, 256 hi columns then 256 lo
    columns of the scaled entries (hi = fp8(x*S), lo = fp8(x*S - hi)).
    """
    blocks = {}
    rows_b = []
    for l in range(3):
        val = np.asarray(params[f"val_emb_{l}"], np.float32)
        dep = np.asarray(params[f"dep_emb_{l}"], np.float32)
        pe = np.asarray(params[f"pos_emb_{l}"], np.float32)
        rows_b.append(val[1:4])
        for s in range(3):
            rows_b.append(pe[s][1:64])
        rows_b.append(dep)  # rows for depth values 0..5, indexed by real depth
    blocks["B"] = np.concatenate(rows_b, 0)
    for name, li in (("L3", 3), ("L4", 4)):
        k = CONV_SIZE[li]
        w = np.asarray(params[f"conv_w_{li}"], np.float32)  # [O, E, k]
        bias = np.asarray(params[f"conv_b_{li}"], np.float32)
        val = np.asarray(params[f"val_emb_{li}"], np.float32)
        pe = np.asarray(params[f"pos_emb_{li}"], np.float32)
        rws = []
        for j in range(k):
            wj = w[:, :, j]
            v = val[1:4] @ wj.T
            if j == 0:
                v = v + bias[None]
            rws.append(v)
            for s in range(3):
                rws.append(pe[s][1:64] @ wj.T)
        blocks[name] = np.concatenate(rws, 0)

    tbl = np.zeros((128, NCH * 2 * E), _FP8)
    for name, _, _, c, nrows in VLAYERS:
        rows = blocks[name]
        assert rows.shape[0] == nrows
        c0 = CHUNK0[name]
        for ci in range(c):
            a = ci * 128
            b = min(a + 128, nrows)
            q = rows[a:b] * TBL_SCALE
            hi = q.astype(_FP8)
            lo = (q - hi.astype(np.float32)).astype(_FP8)
            col = (c0 + ci) * 2 * E
            tbl[: b - a, col : col + E] = hi
            tbl[: b - a, col + E : col + 2 * E] = lo
    return tbl


def _mh_from_rows(rowid, T, c):
    """rowid [T, G] global row ids -> [128, c*T] fp8 multi-hot, chunk-major."""
    mh = np.zeros((c * 128, T), np.float32)
    mh[rowid.T, np.arange(T)[None, :]] = 1.0
    return (
        mh.reshape(c, 128, T).transpose(1, 0, 2).reshape(128, c * T).astype(_FP8)
    )


def _build_mh(value, depth, position, b):
    """Per-core multi-hot selector matrices, one per virtual layer."""
    out = {}
    # B: merged layers 0-2, 198 rows per layer (val 3, pos 3x63, dep 6)
    T = 584
    rowid = np.empty((T, 5), np.int64)
    col, base = 0, 0
    for l in range(3):
        lo, hi = LAYER_SL[l]
        n = hi - lo
        sl = slice(col, col + n)
        rowid[sl, 0] = base + (value[b, lo:hi] - 1)
        for s in range(3):
            rowid[sl, 1 + s] = base + 3 + 63 * s + (position[b, lo:hi, s] - 1)
        rowid[sl, 4] = base + 192 + depth[b, lo:hi]
        col += n
        base += 198
    out["B"] = _mh_from_rows(rowid, T, 5)

    for name, li, c in (("L3", 3, 6), ("L4", 4, 12)):
        k = CONV_SIZE[li]
        lo, hi = LAYER_SL[li]
        T = (hi - lo) // k
        v = value[b, lo:hi].reshape(T, k)
        p = position[b, lo:hi].reshape(T, k, SPATIAL)
        rowid = np.empty((T, 4 * k), np.int64)
        for j in range(k):
            base = 192 * j
            rowid[:, 4 * j] = base + (v[:, j] - 1)
            for s in range(3):
                rowid[:, 4 * j + 1 + s] = base + 3 + 63 * s + (p[:, j, s] - 1)
        out[name] = _mh_from_rows(rowid, T, c)
    return out


_CACHE = {}

PSUM_BUFS = 8
STAGE_TILES = 6  # t-tiles per staging buffer / output store
OB_BUFS = 16  # one staging buffer per store group: stores never backpressure PE


def _stage_groups(ntiles):
    """Tile-group sizes per staging buffer; split the final full group so the
    post-PE drain (evict+store) is short."""
    groups = []
    rem = ntiles
    while rem > 0:
        g = min(STAGE_TILES, rem)
        groups.append(g)
        rem -= g
    if groups and groups[-1] == STAGE_TILES:
        groups[-1] = 2
        groups += [1, 1]
    return groups


def _get_nc(reps=1):
    key = ("v3", PSUM_BUFS, STAGE_TILES, OB_BUFS, L4_BLOCKS, L3_BLOCKS, N_WARMUP, reps)
    if key in _CACHE:
        return _CACHE[key]

    import concourse.bass as bass
    import concourse.tile as tile
    from concourse import bacc, mybir
    from contextlib import ExitStack

    f32 = mybir.dt.float32
    bf16 = mybir.dt.bfloat16
    fp8 = mybir.dt.float8e4

    nc = bacc.Bacc(trn_type="TRN2", target_bir_lowering=False, debug=False)
    tbl_d = nc.dram_tensor("tbl", [128, NCH * 2 * E], fp8, kind="ExternalInput").ap()
    mh_d = {
        name: nc.dram_tensor(f"mh_{name}", [128, c * T], fp8, kind="ExternalInput").ap()
        for name, T, _, c, _ in VLAYERS
    }
    out_d = nc.dram_tensor("out", [OUT_TOKENS, E], bf16, kind="ExternalOutput").ap()

    with tile.TileContext(nc) as tc, ExitStack() as ctx:
        cpool = ctx.enter_context(tc.tile_pool(name="const", bufs=1))
        bps = ctx.enter_context(
            tc.tile_pool(name="bps", bufs=PSUM_BUFS, space=bass.MemorySpace.PSUM)
        )
        opool = ctx.enter_context(tc.tile_pool(name="osb", bufs=OB_BUFS))
        wpool = ctx.enter_context(tc.tile_pool(name="warm", bufs=1))

        tbl_t = cpool.tile([128, NCH * 2 * E], fp8, tag="tbl")
        mh_t = {
            name: cpool.tile(
                [128, c * T], fp8, tag=f"mh{name}", name=f"mh{name}_t"
            )
            for name, T, _, c, _ in VLAYERS
        }

        A = mybir.ActivationFunctionType

        # Warmup: keep the PE busy through the initial DMA latency so the
        # p-state ramp completes before real work starts.
        wm = wpool.tile([128, 128], fp8, tag="wm")
        wt = wpool.tile([128, E], bf16, tag="wt")
        nc.vector.memset(wm[:], 0.0)
        nc.vector.memset(wt[:], 0.0)
        wp = bps.tile([128, E], f32, tag="ps")
        for _ in range(N_WARMUP):
            nc.tensor.matmul(wp[:], wm[:], wt[:], start=True, stop=True)

        # DMA order drives availability: B consts, then L3, then L4 in blocks.
        nc.sync.dma_start(tbl_t[:, : 10 * E], tbl_d[:, : 10 * E])
        nc.sync.dma_start(mh_t["B"][:], mh_d["B"][:])
        nc.sync.dma_start(tbl_t[:, 10 * E : 22 * E], tbl_d[:, 10 * E : 22 * E])
        blk3 = 1024 // L3_BLOCKS
        src3 = mh_d["L3"][:].rearrange("p (c t) -> p c t", c=6)
        dst3 = mh_t["L3"][:].rearrange("p (c t) -> p c t", c=6)
        for bi in range(L3_BLOCKS):
            nc.sync.dma_start(
                dst3[:, :, bi * blk3 : (bi + 1) * blk3],
                src3[:, :, bi * blk3 : (bi + 1) * blk3],
            )
        nc.sync.dma_start(tbl_t[:, 22 * E :], tbl_d[:, 22 * E :])
        blk = 4096 // L4_BLOCKS
        src4 = mh_d["L4"][:].rearrange("p (c t) -> p c t", c=12)
        dst4 = mh_t["L4"][:].rearrange("p (c t) -> p c t", c=12)
        for bi in range(L4_BLOCKS):
            nc.sync.dma_start(
                dst4[:, :, bi * blk : (bi + 1) * blk],
                src4[:, :, bi * blk : (bi + 1) * blk],
            )

        def emit_body():
            items = []
            for name, T, out_off, c, _ in VLAYERS:
                g0 = 0
                for gn in _stage_groups(-(-T // 128)):
                    items.append((name, T, out_off, c, g0, gn))
                    g0 += gn
            for name, T, out_off, c, g0, gn in items:
                c0 = CHUNK0[name]
                if True:
                    ob = opool.tile([128, gn * E], bf16, tag="ob")
                    for h in range(gn):
                        t0 = (g0 + h) * 128
                        M = min(128, T - t0)
                        ps = bps.tile([128, E], f32, tag="ps")
                        for ci in range(c):
                            lhsT = (
                                mh_t[name][:, ci * T + t0 : ci * T + t0 + M]
                                .unsqueeze(1)
                                .broadcast_to([128, 2, M])
                            )
                            col = (c0 + ci) * 2 * E
                            rhs = tbl_t[:, col : col + 2 * E].rearrange(
                                "p (i n) -> p i n", i=2
                            )
                            nc.tensor.matmul(
                                ps[:M, :],
                                lhsT,
                                rhs,
                                start=(ci == 0),
                                stop=(ci == c - 1),
                                perf_mode=mybir.MatmulPerfMode.DoubleRow,
                            )
                        nc.scalar.activation(
                            ob[:M, h * E : (h + 1) * E],
                            ps[:M, :],
                            A.Copy,
                            scale=1.0 / TBL_SCALE,
                        )
                    row = out_off + g0 * 128
                    W = min(T - g0 * 128, gn * 128)
                    if W % 128 == 0:
                        dst = out_d[row : row + W, :].rearrange(
                            "(a p) e -> p a e", p=128
                        )
                        src = ob[:].rearrange("p (a e) -> p a e", e=E)
                        nc.sync.dma_start(dst, src)
                    else:
                        # ragged tail (B: 72 tokens)
                        full = W // 128
                        if full:
                            dst = out_d[row : row + full * 128, :].rearrange(
                                "(a p) e -> p a e", p=128
                            )
                            src = ob[:, : full * E].rearrange(
                                "p (a e) -> p a e", e=E
                            )
                            nc.sync.dma_start(dst, src)
                        rem = W - full * 128
                        nc.sync.dma_start(
                            out_d[row + full * 128 : row + W, :],
                            ob[:rem, full * E : full * E + E],
                        )

        if reps == 1:
            emit_body()
        else:
            hints = (
                mybir.EngineType.PE,
                mybir.EngineType.Activation,
                mybir.EngineType.SP,
            )
            with tc.For_i(0, reps, 1, hint_engines=hints):
                emit_body()

    nc.compile()
    _CACHE[key] = nc
    return nc


def kernel(**inputs):
    from concourse.bass_utils import run_bass_kernel_spmd

    value = np.asarray(inputs["value"], np.int64)
    depth = np.asarray(inputs["depth"], np.int64)
    position = np.asarray(inputs["position"], np.int64)
    # row-id arithmetic relies on the generator's index ranges
    assert value.min() >= 1 and value.max() <= NUM_VOCAB
    assert position.min() >= 1 and position.max() < 2 * RES
    assert depth.min() >= 0 and depth.max() <= 5

    tbl = _build_tables(inputs)
    nc = _get_nc()

    in_maps = []
    for b in range(BATCH):
        mh = _build_mh(value, depth, position, b)
        m = {"tbl": tbl}
        for name, _, _, _, _ in VLAYERS:
            m[f"mh_{name}"] = mh[name]
        in_maps.append(m)

    res = run_bass_kernel_spmd(nc, in_maps, list(range(BATCH)))
    return np.stack(
        [res.results[b]["out"] for b in range(BATCH)]
    ).astype(np.float32)
